# revision 1
# baseline (speedup 1.0000x reference)
"""FlexibleThresholdedLoss on 8 Trainium2 NeuronCores.

Strategy (pure data parallel over the batch dim):
  - Each core gets 4 of the 32 images of both inputs, viewed as [128, 24576] f32.
  - Phase A (streaming from HBM, DMA-bound):
      d = a - b (DVE); u = |d| fp16 and q = d^2 fp16 resident in SBUF (ACT);
      global sums of u and q via PE ones-matmuls accumulating in PSUM.
  - One AllReduce of the two scalar partials -> global mae/mse means (t, s).
  - Phase B (on SBUF-resident u and q, no HBM traffic), in-domain max algebra:
      c1 = #[u >= t],  Sm1 = sum max(u, t)   (mae side)
      c2 = #[q >= s],  Sm2 = sum max(q, s)   (mse side, squared domain)
    masks/max tiles via DVE tensor_scalar (4x fp16), sums via PE ones-matmuls.
  - Host closes the algebra exactly:
      s1 = Sm1 - t*(N - c1)     (masked |d| sum;  mae_thr = s1/c1)
      s2 = Sm2 - s*(N - c2)     (masked d^2 sum;  mse_thr = s2/c2)
"""

import numpy as np

import concourse.bacc as bacc
import concourse.mybir as mybir
from concourse import bass_isa
from concourse.bass_utils import run_bass_kernel_spmd
from concourse.tile import TileContext

P = 128
FD = 24576            # per-partition elements per input tensor (per core)
N_CORES = 8
N_TOTAL = 32 * 3 * 512 * 512   # 25_165_824 global element count
CHUNK_A = 1024        # phase A chunk (24 chunks, 1 MB DMA each, 3 per HWDGE queue)
CHUNK_B = 4096        # phase B chunk (6 chunks)
MM_N = 512            # PE ones-matmul moving free dim (one PSUM bank)

F32 = mybir.dt.float32
F16 = mybir.dt.float16
ALU = mybir.AluOpType
ACTF = mybir.ActivationFunctionType
AX = mybir.AxisListType

N_A = FD // CHUNK_A   # 12
N_B = FD // CHUNK_B   # 6

# ablation controls for the timing loop (None = full)
LOOP_PARTS_A = None
LOOP_PARTS_B = None

PARTS_A = frozenset({"dma", "sub", "abs", "square", "pe_u", "pe_q"})
PARTS_B = frozenset({"m1", "c1", "m2", "c2"})


def _pe_sum(nc, ones, psum, src, start, stop=False):
    """Accumulate per-column partition sums of src [P, W] into psum [1, MM_N]."""
    w = src.shape[-1]
    for j in range(0, w, MM_N):
        nc.tensor.matmul(
            psum[:, 0:MM_N],
            ones[:, 0:1],
            src[:, j : j + MM_N],
            start=(start and j == 0),
            stop=(stop and j + MM_N >= w),
        )


def _emit_phase_a(nc, ones, ab_d, u, q, psum_u, psum_q, stage_pool, d_pool,
                  parts=PARTS_A, first=True):
    for c in range(N_A):
        sl = slice(c * CHUNK_A, (c + 1) * CHUNK_A)
        if "dma" not in parts:
            continue
        abt = stage_pool.tile([P, 2 * CHUNK_A], F32, tag="stage")
        nc.sync.dma_start(
            abt[:], ab_d[:, 2 * c * CHUNK_A : 2 * (c + 1) * CHUNK_A]
        )
        if "sub" not in parts:
            continue
        dt_ = d_pool.tile([P, CHUNK_A], F32, tag="d")
        nc.vector.tensor_tensor(
            dt_[:], abt[:, 0:CHUNK_A], abt[:, CHUNK_A : 2 * CHUNK_A],
            op=ALU.subtract,
        )
        if "abs" in parts:
            nc.scalar.activation(u[:, sl], dt_[:], ACTF.Abs)
        if "square" in parts:
            nc.scalar.activation(q[:, sl], dt_[:], ACTF.Square)
        if "pe_u" in parts:
            _pe_sum(nc, ones, psum_u, u[:, sl], start=(first and c == 0),
                    stop=(c == N_A - 1))
        if "pe_q" in parts:
            _pe_sum(nc, ones, psum_q, q[:, sl], start=(first and c == 0),
                    stop=(c == N_A - 1))


def _emit_phase_b(nc, ones, u, q, thr, psums, stage_pool, parts=PARTS_B,
                  first=True):
    t_ap = thr[:, 0:1]
    s_ap = thr[:, 1:2]
    specs = [
        ("m1", u, t_ap, ALU.max, "pm1"),
        ("c1", u, t_ap, ALU.is_ge, "pc1"),
        ("m2", q, s_ap, ALU.max, "pm2"),
        ("c2", q, s_ap, ALU.is_ge, "pc2"),
    ]
    for k in range(N_B):
        sl = slice(k * CHUNK_B, (k + 1) * CHUNK_B)
        for name, buf, scal, op, pname in specs:
            if name not in parts:
                continue
            ot = stage_pool.tile([P, CHUNK_B], F16, tag="stage")
            nc.vector.tensor_scalar(ot[:], buf[:, sl], scal, None, op0=op)
            _pe_sum(nc, ones, psums[pname], ot[:], start=(first and k == 0),
                    stop=(k == N_B - 1))


def _build_program(stop_after="full", loop_n=0, loop_cc=False):
    nc = bacc.Bacc("TRN2", target_bir_lowering=False)

    # host packs a and b interleaved per chunk: [P, n_chunks, 2, CHUNK_A]
    ab_d = nc.declare_dram_parameter("ab", [P, 2 * FD], F32, isOutput=False)
    out_d = nc.declare_dram_parameter("partials", [1, 16], F32, isOutput=True)

    with TileContext(nc) as tc:
        with (
            tc.tile_pool(name="ubuf", bufs=1) as ubuf_pool,
            tc.tile_pool(name="stage", bufs=8) as stage_pool,
            tc.tile_pool(name="dbuf", bufs=3) as d_pool,
            tc.tile_pool(name="small", bufs=1) as small_pool,
            tc.tile_pool(name="psum", bufs=1, space="PSUM") as psum_pool,
            tc.tile_pool(name="dram", bufs=1, space="DRAM") as dram_pool,
        ):
            u = ubuf_pool.tile([P, FD], F16, tag="u")
            q = ubuf_pool.tile([P, FD], F16, tag="q")

            ones = small_pool.tile([P, 1], F16, tag="ones")
            nc.vector.memset(ones[:], 1.0)

            psum_u = psum_pool.tile([1, MM_N], F32, tag="pu")
            psum_q = psum_pool.tile([1, MM_N], F32, tag="pq")
            psums = {
                n: psum_pool.tile([1, MM_N], F32, tag=n, name=n)
                for n in ("pm1", "pc1", "pm2", "pc2")
            }

            _emit_phase_a(
                nc, ones, ab_d, u, q, psum_u, psum_q, stage_pool, d_pool
            )

            # ---- core-local scalars, all-reduce across cores ----
            sums2 = small_pool.tile([1, 2], F32, tag="sums2")
            nc.vector.tensor_reduce(
                sums2[:, 0:1], psum_u[:, :], axis=AX.X, op=ALU.add
            )
            nc.vector.tensor_reduce(
                sums2[:, 1:2], psum_q[:, :], axis=AX.X, op=ALU.add
            )

            cc_in = dram_pool.tile([1, 2], F32, tag="cc_in")
            cc_out = dram_pool.tile([1, 2], F32, tag="cc_out", addr_space="Shared")
            nc.sync.dma_start(cc_in[:], sums2[:])
            nc.gpsimd.collective_compute(
                "AllReduce",
                ALU.add,
                replica_groups=[list(range(N_CORES))],
                ins=[cc_in.opt()],
                outs=[cc_out.opt()],
            )
            g = small_pool.tile([1, 2], F32, tag="g")
            nc.sync.dma_start(g[:], cc_out[:])

            # thresholds: t = mae mean, s = mse mean (both on partition 0)
            ts2 = small_pool.tile([1, 2], F32, tag="ts2")
            inv_n = 1.0 / float(N_TOTAL)
            nc.scalar.mul(ts2[:, 0:2], g[:, 0:2], inv_n)
            thr = small_pool.tile([P, 2], F32, tag="thr")
            nc.gpsimd.partition_broadcast(thr[:], ts2[:], channels=P)

            if stop_after == "thresholds":
                dbg = small_pool.tile([1, 16], F32, tag="dbg")
                nc.vector.memset(dbg[:], 0.0)
                nc.scalar.copy(dbg[:, 0:2], g[:, 0:2])
                nc.scalar.copy(dbg[:, 2:4], ts2[:, 0:2])
                nc.sync.dma_start(out_d[:], dbg[:])
            else:
                _emit_phase_b(nc, ones, u, q, thr, psums, stage_pool)

                # ---- final reductions + output row ----
                outrow = small_pool.tile([1, 16], F32, tag="outrow")
                nc.vector.memset(outrow[:], 0.0)
                nc.scalar.copy(outrow[:, 0:2], g[:, 0:2])     # G_u, G_q
                nc.scalar.copy(outrow[:, 2:4], ts2[:, 0:2])   # t, s
                for j, pname in enumerate(("pc1", "pm1", "pc2", "pm2")):
                    nc.vector.tensor_reduce(
                        outrow[:, 4 + j : 5 + j], psums[pname][:, :],
                        axis=AX.X, op=ALU.add,
                    )
                nc.sync.dma_start(out_d[:], outrow[:])

                if loop_n:
                    # timing rig: repeat the A+B workload loop_n more times
                    pa = LOOP_PARTS_A if LOOP_PARTS_A is not None else PARTS_A
                    pb = LOOP_PARTS_B if LOOP_PARTS_B is not None else PARTS_B
                    if loop_cc:
                        cc_in2 = dram_pool.tile([1, 2], F32, tag="cc_in2",
                                                name="cc_in2")
                        cc_out2 = dram_pool.tile([1, 2], F32, tag="cc_out2",
                                                 name="cc_out2",
                                                 addr_space="Shared")
                    with tc.For_i(0, loop_n, 1):
                        _emit_phase_a(
                            nc, ones, ab_d, u, q, psum_u, psum_q, stage_pool,
                            d_pool, parts=pa, first=True,
                        )
                        if loop_cc:
                            nc.vector.tensor_reduce(
                                sums2[:, 0:1], psum_u[:, :], axis=AX.X,
                                op=ALU.add,
                            )
                            nc.vector.tensor_reduce(
                                sums2[:, 1:2], psum_q[:, :], axis=AX.X,
                                op=ALU.add,
                            )
                            nc.sync.dma_start(cc_in2[:], sums2[:])
                            nc.gpsimd.collective_compute(
                                "AllReduce",
                                ALU.add,
                                replica_groups=[list(range(N_CORES))],
                                ins=[cc_in2.opt()],
                                outs=[cc_out2.opt()],
                            )
                            nc.sync.dma_start(g[:], cc_out2[:])
                            nc.scalar.mul(ts2[:, 0:2], g[:, 0:2], inv_n)
                            nc.gpsimd.partition_broadcast(
                                thr[:], ts2[:], channels=P
                            )
                        if pb:
                            _emit_phase_b(
                                nc, ones, u, q, thr, psums, stage_pool,
                                parts=pb, first=True,
                            )

    nc.compile()
    return nc


_NC_CACHE = None


def _get_program():
    global _NC_CACHE
    if _NC_CACHE is None:
        _NC_CACHE = _build_program()
    return _NC_CACHE


def _shard_inputs(input_img: np.ndarray, target_img: np.ndarray):
    a = np.asarray(input_img, dtype=np.float32)
    b = np.asarray(target_img, dtype=np.float32)
    per = a.shape[0] // N_CORES
    in_maps = []
    for i in range(N_CORES):
        sl = slice(i * per, (i + 1) * per)
        ai = np.ascontiguousarray(a[sl]).reshape(P, N_A, 1, CHUNK_A)
        bi = np.ascontiguousarray(b[sl]).reshape(P, N_A, 1, CHUNK_A)
        # interleave per chunk: [P, n_a, 2, CHUNK_A] -> [P, 2*FD]
        ab = np.concatenate([ai, bi], axis=2).reshape(P, 2 * FD)
        in_maps.append({"ab": np.ascontiguousarray(ab)})
    return in_maps


def _combine(results) -> np.float32:
    # identical on every core: global sums + thresholds
    row0 = results[0]["partials"].reshape(-1).astype(np.float64)
    g_u, g_q, t, s = row0[0], row0[1], row0[2], row0[3]
    # per-core masked partials: sum over cores
    c1 = sm1 = c2 = sm2 = 0.0
    for res in results:
        row = res["partials"].reshape(-1).astype(np.float64)
        c1 += row[4]
        sm1 += row[5]
        c2 += row[6]
        sm2 += row[7]

    n = float(N_TOTAL)
    mae_loss = g_u / n
    mse_loss = g_q / n

    s1 = sm1 - t * (n - c1)   # sum u over u >= t
    s2 = sm2 - s * (n - c2)   # sum q over q >= s

    mae_thr = s1 / c1 if c1 > 0 else 0.0
    mse_thr = s2 / c2 if c2 > 0 else 0.0

    combined_thr = 0.5 * mae_thr + 0.5 * mse_thr
    combined_non = 0.5 * mae_loss + 0.5 * mse_loss
    total = 0.5 * combined_thr + 0.5 * combined_non
    return np.float32(total)


def kernel(input_img: np.ndarray, target_img: np.ndarray) -> np.ndarray:
    import time as _time

    nc = _get_program()
    in_maps = _shard_inputs(input_img, target_img)
    last_err = None
    for attempt in range(3):
        try:
            res = run_bass_kernel_spmd(nc, in_maps, list(range(N_CORES)))
            return np.asarray(_combine(res.results))
        except Exception as e:  # transient device-unrecoverable states
            last_err = e
            _time.sleep(20 * (attempt + 1))
    raise last_err



# revision 4
# speedup vs baseline: 3.3265x; 3.3265x over previous
"""FlexibleThresholdedLoss on 8 Trainium2 NeuronCores.

Strategy (pure data parallel over the batch dim): each core gets 4 of the
32 images of both inputs, viewed as [128, 24576] f32 per tensor.

Phase A (streams 24 MiB f32 from HBM per core; DMA is the binding
resource at ~420 GB/s into SBUF):
  - 12 x 2 MiB HWDGE DMAs of interleaved (a|b) chunks into a deep stage
    pool (bufs=6) to keep the SDMA queue saturated.
  - d = a - b in fp16, column-split between GPSIMD (slow but idle) and
    DVE so no single engine exceeds the DMA shadow. d stays resident.
  - ACT Abs(d)+accum -> S_u partials; ACT Square(d)+accum -> S_q
    partials (both activation outputs are discarded scratch; ACT's
    fused accumulator does the reduction for free).
One AllReduce of (S_u, S_q) -> global means t (mae) and s (mse);
thresholds t^2 and s are broadcast to all partitions.

Phase B (SBUF-resident d only, everything in the q = d^2 domain):
  - q_c = ACT Square(d_c) per 4096-col chunk.
  - M1 = sum sqrt(max(q, t^2)) : DVE max -> ACT Sqrt with accum.
  - C1 = #[q >= t^2], C2 = #[q >= s], M2 = sum max(q, s): DVE
    tensor_scalar outputs summed by PE ones-matmuls into PSUM banks.
Host closes the algebra exactly:
  s1 = M1 - t*(N - C1)   (masked |d| sum;  mae_thr = s1/C1)
  s2 = M2 - s*(N - C2)   (masked d^2 sum;  mse_thr = s2/C2)
"""

import numpy as np

import concourse.bacc as bacc
import concourse.mybir as mybir
from concourse.bass_utils import run_bass_kernel_spmd
from concourse.tile import TileContext

P = 128
FD = 24576            # per-partition elements per input tensor (per core)
N_CORES = 8
N_TOTAL = 32 * 3 * 512 * 512   # 25_165_824 global element count

CHUNK = 2048          # f32 cols per tensor per DMA chunk (2 MiB DMAs)
N_A = FD // CHUNK     # 12 DMA chunks
STAGE_BUFS = 6

BLK = 4096            # compute block (2 DMA chunks)
N_BLK = FD // BLK     # 6
GP_BLKS = 4           # subtract blocks owned by GPSIMD (rest on DVE)
DVE_COLS = (N_BLK - GP_BLKS) * BLK
GP_COLS = GP_BLKS * BLK

MM_N = 512            # PE ones-matmul free-dim slice (one PSUM bank col set)

F32 = mybir.dt.float32
F16 = mybir.dt.float16
ALU = mybir.AluOpType
ACTF = mybir.ActivationFunctionType
AX = mybir.AxisListType

# ablation controls for the timing loop (None = full)
LOOP_PARTS_A = None
LOOP_PARTS_B = None
PARTS_A = frozenset({"dma", "sub", "abs", "sq"})
PARTS_B = frozenset({"qc", "m1", "c1", "m2", "c2"})


def _pe_sum(nc, ones, psum, src, start, stop=False):
    w = src.shape[-1]
    for j in range(0, w, MM_N):
        nc.tensor.matmul(
            psum[:, 0:MM_N],
            ones[:, 0:1],
            src[:, j : j + MM_N],
            start=(start and j == 0),
            stop=(stop and j + MM_N >= w),
        )


def _emit_phase_a(nc, ab_d, d0, d1, accu, accq, stage_pool, scra_pool,
                  parts=PARTS_A):
    """DMA + subtract + |d|/d^2 accumulation. d0 holds DVE-computed
    columns [0, DVE_COLS), d1 holds GPSIMD columns [DVE_COLS, FD)."""
    stages = []
    for c in range(N_A):
        if "dma" not in parts:
            continue
        st = stage_pool.tile([P, 2 * CHUNK], F32, tag="stage")
        nc.sync.dma_start(
            st[:], ab_d[:, 2 * c * CHUNK : 2 * (c + 1) * CHUNK]
        )
        stages.append(st)
        # process one BLK (= 2 chunks) when its pair of stages is in
        if "sub" not in parts or c % 2 == 0:
            continue
        b = c // 2          # block index 0..N_BLK-1
        st0, st1 = stages[-2], stages[-1]
        col = b * BLK
        if col < DVE_COLS:
            dt_, dcol, eng = d0, col, nc.vector
        else:
            dt_, dcol, eng = d1, col - DVE_COLS, nc.gpsimd
        for half, sh in ((0, st0), (1, st1)):
            eng.tensor_tensor(
                dt_[:, dcol + half * CHUNK : dcol + (half + 1) * CHUNK],
                sh[:, 0:CHUNK],
                sh[:, CHUNK : 2 * CHUNK],
                op=ALU.subtract,
            )
        dv = dt_[:, dcol : dcol + BLK]
        if "abs" in parts:
            scr = scra_pool.tile([P, BLK], F16, tag="scra")
            nc.scalar.activation(scr[:], dv, ACTF.Abs,
                                 accum_out=accu[:, b : b + 1])
        if "sq" in parts:
            scr = scra_pool.tile([P, BLK], F16, tag="scra")
            nc.scalar.activation(scr[:], dv, ACTF.Square,
                                 accum_out=accq[:, b : b + 1])


def _emit_phase_b(nc, ones, d0, d1, thr, psums, accm1, qc_pool, scrb_pool,
                  parts=PARTS_B):
    """Thresholded reductions in the q domain. thr[:,0:1]=t^2, thr[:,1:2]=s."""
    t2 = thr[:, 0:1]
    s_ = thr[:, 1:2]
    for b in range(N_BLK):
        col = b * BLK
        if col < DVE_COLS:
            dv = d0[:, col : col + BLK]
        else:
            dv = d1[:, col - DVE_COLS : col - DVE_COLS + BLK]
        if "qc" not in parts:
            continue
        qc = qc_pool.tile([P, BLK], F16, tag="qc")
        nc.scalar.activation(qc[:], dv, ACTF.Square)
        if "m1" in parts:
            mx = scrb_pool.tile([P, BLK], F16, tag="scrb")
            nc.vector.tensor_scalar(mx[:], qc[:], t2, None, op0=ALU.max)
            scr = scrb_pool.tile([P, BLK], F16, tag="scrb")
            nc.scalar.activation(scr[:], mx[:], ACTF.Sqrt,
                                 accum_out=accm1[:, b : b + 1])
        for name, scal, op, pname in (
            ("c1", t2, ALU.is_ge, "pc1"),
            ("m2", s_, ALU.max, "pm2"),
            ("c2", s_, ALU.is_ge, "pc2"),
        ):
            if name not in parts:
                continue
            ot = scrb_pool.tile([P, BLK], F16, tag="scrb")
            nc.vector.tensor_scalar(ot[:], qc[:], scal, None, op0=op)
            _pe_sum(nc, ones, psums[pname], ot[:], start=(b == 0),
                    stop=(b == N_BLK - 1))


def _build_program(loop_n=0):
    nc = bacc.Bacc("TRN2", target_bir_lowering=False)

    # host packs a and b interleaved per chunk: [P, n_a, 2, CHUNK] f32
    ab_d = nc.declare_dram_parameter("ab", [P, 2 * FD], F32, isOutput=False)
    out_d = nc.declare_dram_parameter("partials", [1, 16], F32, isOutput=True)

    with TileContext(nc) as tc:
        with (
            tc.tile_pool(name="stage", bufs=STAGE_BUFS) as stage_pool,
            tc.tile_pool(name="dres", bufs=1) as dres_pool,
            tc.tile_pool(name="scra", bufs=2) as scra_pool,
            tc.tile_pool(name="qc", bufs=2) as qc_pool,
            tc.tile_pool(name="scrb", bufs=3) as scrb_pool,
            tc.tile_pool(name="small", bufs=1) as small_pool,
            tc.tile_pool(name="psum", bufs=1, space="PSUM") as psum_pool,
            tc.tile_pool(name="dram", bufs=1, space="DRAM") as dram_pool,
        ):
            d0 = dres_pool.tile([P, DVE_COLS], F16, tag="d0")
            d1 = dres_pool.tile([P, GP_COLS], F16, tag="d1")

            ones = small_pool.tile([P, 1], F16, tag="ones")
            nc.vector.memset(ones[:], 1.0)
            ones32 = small_pool.tile([P, 1], F32, tag="ones32")
            nc.vector.memset(ones32[:], 1.0)

            accu = small_pool.tile([P, N_BLK], F32, tag="accu")
            accq = small_pool.tile([P, N_BLK], F32, tag="accq")
            accm1 = small_pool.tile([P, N_BLK], F32, tag="accm1")

            psums = {
                n: psum_pool.tile([1, MM_N], F32, tag=n, name=n)
                for n in ("pc1", "pm2", "pc2")
            }
            psg = psum_pool.tile([1, 2], F32, tag="psg", name="psg")
            psm1 = psum_pool.tile([1, 1], F32, tag="psm1", name="psm1")

            _emit_phase_a(nc, ab_d, d0, d1, accu, accq, stage_pool, scra_pool)

            # ---- core-local scalars, all-reduce across cores ----
            uq = small_pool.tile([P, 2], F32, tag="uq")
            nc.vector.tensor_reduce(uq[:, 0:1], accu[:, :], axis=AX.X,
                                    op=ALU.add)
            nc.vector.tensor_reduce(uq[:, 1:2], accq[:, :], axis=AX.X,
                                    op=ALU.add)
            nc.tensor.matmul(psg[:, 0:2], ones32[:, 0:1], uq[:, 0:2],
                             start=True, stop=True)
            sums2 = small_pool.tile([1, 2], F32, tag="sums2")
            nc.vector.tensor_copy(sums2[:], psg[:, 0:2])

            cc_in = dram_pool.tile([1, 2], F32, tag="cc_in")
            cc_out = dram_pool.tile([1, 2], F32, tag="cc_out",
                                    addr_space="Shared")
            nc.sync.dma_start(cc_in[:], sums2[:])
            nc.gpsimd.collective_compute(
                "AllReduce",
                ALU.add,
                replica_groups=[list(range(N_CORES))],
                ins=[cc_in.opt()],
                outs=[cc_out.opt()],
            )
            g = small_pool.tile([1, 2], F32, tag="g")
            nc.sync.dma_start(g[:], cc_out[:])

            # thresholds: t = mae mean, s = mse mean; need (t^2, s) on all
            # partitions.
            ts2 = small_pool.tile([1, 2], F32, tag="ts2")
            inv_n = 1.0 / float(N_TOTAL)
            nc.scalar.mul(ts2[:, 0:2], g[:, 0:2], inv_n)
            th = small_pool.tile([1, 2], F32, tag="th")
            nc.scalar.activation(th[:, 0:1], ts2[:, 0:1], ACTF.Square)
            nc.scalar.copy(th[:, 1:2], ts2[:, 1:2])
            thr = small_pool.tile([P, 2], F32, tag="thr")
            nc.gpsimd.partition_broadcast(thr[:], th[:], channels=P)

            _emit_phase_b(nc, ones, d0, d1, thr, psums, accm1, qc_pool,
                          scrb_pool)

            # ---- final reductions + output row ----
            m1p = small_pool.tile([P, 1], F32, tag="m1p")
            nc.vector.tensor_reduce(m1p[:, 0:1], accm1[:, :], axis=AX.X,
                                    op=ALU.add)
            nc.tensor.matmul(psm1[:, 0:1], ones32[:, 0:1], m1p[:, 0:1],
                             start=True, stop=True)

            outrow = small_pool.tile([1, 16], F32, tag="outrow")
            nc.vector.memset(outrow[:], 0.0)
            nc.scalar.copy(outrow[:, 0:2], g[:, 0:2])     # G_u, G_q
            nc.scalar.copy(outrow[:, 2:4], ts2[:, 0:2])   # t, s
            for j, pname in enumerate(("pc1", "pm2", "pc2")):
                nc.vector.tensor_reduce(
                    outrow[:, 4 + j : 5 + j], psums[pname][:, :],
                    axis=AX.X, op=ALU.add,
                )
            nc.scalar.copy(outrow[:, 7:8], psm1[:, 0:1])  # M1
            nc.sync.dma_start(out_d[:], outrow[:])

            if loop_n:
                pa = LOOP_PARTS_A if LOOP_PARTS_A is not None else PARTS_A
                pb = LOOP_PARTS_B if LOOP_PARTS_B is not None else PARTS_B
                with tc.For_i(0, loop_n, 1):
                    _emit_phase_a(nc, ab_d, d0, d1, accu, accq, stage_pool,
                                  scra_pool, parts=pa)
                    if pb:
                        _emit_phase_b(nc, ones, d0, d1, thr, psums, accm1,
                                      qc_pool, scrb_pool, parts=pb)

    nc.compile()
    return nc


_NC_CACHE = None


def _get_program():
    global _NC_CACHE
    if _NC_CACHE is None:
        _NC_CACHE = _build_program()
    return _NC_CACHE


def _shard_inputs(input_img: np.ndarray, target_img: np.ndarray):
    a = np.asarray(input_img, dtype=np.float32)
    b = np.asarray(target_img, dtype=np.float32)
    per = a.shape[0] // N_CORES
    in_maps = []
    for i in range(N_CORES):
        sl = slice(i * per, (i + 1) * per)
        ai = np.ascontiguousarray(a[sl]).reshape(P, N_A, 1, CHUNK)
        bi = np.ascontiguousarray(b[sl]).reshape(P, N_A, 1, CHUNK)
        ab = np.concatenate([ai, bi], axis=2).reshape(P, 2 * FD)
        in_maps.append({"ab": np.ascontiguousarray(ab)})
    return in_maps


def _combine(results) -> np.float32:
    # identical on every core: global sums + thresholds
    row0 = results[0]["partials"].reshape(-1).astype(np.float64)
    g_u, g_q, t, s = row0[0], row0[1], row0[2], row0[3]
    c1 = m2 = c2 = m1 = 0.0
    for res in results:
        row = res["partials"].reshape(-1).astype(np.float64)
        c1 += row[4]
        m2 += row[5]
        c2 += row[6]
        m1 += row[7]

    n = float(N_TOTAL)
    mae_loss = g_u / n
    mse_loss = g_q / n

    s1 = m1 - t * (n - c1)   # sum |d| over q >= t^2
    s2 = m2 - s * (n - c2)   # sum d^2 over q >= s

    mae_thr = s1 / c1 if c1 > 0 else 0.0
    mse_thr = s2 / c2 if c2 > 0 else 0.0

    combined_thr = 0.5 * mae_thr + 0.5 * mse_thr
    combined_non = 0.5 * mae_loss + 0.5 * mse_loss
    total = 0.5 * combined_thr + 0.5 * combined_non
    return np.float32(total)


def kernel(input_img: np.ndarray, target_img: np.ndarray) -> np.ndarray:
    import time as _time

    nc = _get_program()
    in_maps = _shard_inputs(input_img, target_img)
    last_err = None
    for attempt in range(3):
        try:
            res = run_bass_kernel_spmd(nc, in_maps, list(range(N_CORES)))
            return np.asarray(_combine(res.results))
        except Exception as e:  # transient device-unrecoverable states
            last_err = e
            _time.sleep(20 * (attempt + 1))
    raise last_err
